# revision 12
# baseline (speedup 1.0000x reference)
"""Trainium2 Bass kernel for AttnNoProjVal.

Computes, per batch element b (one NeuronCore per batch element, B=8):
  q = hs @ Wq.T + bq ; k = hs @ Wk.T + bk
  scoresT[kp, qp] = k[kp] . q[qp] / sqrt(H)      (transposed orientation)
  E = exp(scoresT) * keep_mask[kp]               (no row-max: |scores| <~ 8)
  out[qp, :] = (E.T @ hs) / colsum(E)[qp]

The transposed score orientation puts the softmax reduction on the PE
partition axis, where it folds into the attention-value matmul as an extra
N=1 column of ones -- no transposes and no vector-engine reductions needed.
All matmuls run as float32r (full PE rate at moving-dim >= 256); the
attention-value matmul runs in bf16.
"""

import sys

sys.path.insert(0, "/opt/trn_rl_repo")

from contextlib import ExitStack

import ml_dtypes
import numpy as np

import concourse.bass as bass
import concourse.tile as tile
from concourse import bacc, mybir
from concourse.bass_utils import run_bass_kernel_spmd

B, S, H = 8, 2048, 1024
N_CORES = 8
HC = H // 128   # 8 chunks of the hidden/head dim
SC = S // 128   # 16 chunks of the sequence dim
SB = S // 512   # 4 moving-dim blocks of the sequence dim
F32 = mybir.dt.float32
F32R = mybir.dt.float32r
BF16 = mybir.dt.bfloat16

_CACHED_NC = None


def _r(ap):
    return ap.bitcast(F32R)


def build_nc():
    nc = bacc.Bacc(None, target_bir_lowering=False)

    hsT = nc.dram_tensor("hst", [H, S], F32R, kind="ExternalInput")
    hsb = nc.dram_tensor("hsb", [S, H], BF16, kind="ExternalInput")
    wqT = nc.dram_tensor("wqt", [H, H], F32R, kind="ExternalInput")
    wkT = nc.dram_tensor("wkt", [H, H], F32R, kind="ExternalInput")
    bq = nc.dram_tensor("bq", [H], F32, kind="ExternalInput")
    bk = nc.dram_tensor("bk", [H], F32, kind="ExternalInput")
    mk = nc.dram_tensor("mk", [S], F32, kind="ExternalInput")
    out = nc.dram_tensor("out", [S, H], F32, kind="ExternalOutput")
    # Per-block DRAM spill tensors for q^T: block b's reload in phase B only
    # depends on its own 8 spill writes, so it overlaps the rest of phase A.
    qts = [nc.dram_tensor(f"qts{b}", [H, 512], F32R) for b in range(SB)]

    with tile.TileContext(nc) as tc, ExitStack() as whole:
        singles = whole.enter_context(tc.tile_pool(name="singles", bufs=1))
        kt_pool = whole.enter_context(tc.tile_pool(name="ktp", bufs=1))

        junk = singles.tile([128, 512], BF16, tag="junk", name="junk")
        nc.vector.memset(junk[:], 0.0)
        bq_sb = singles.tile([128, HC], F32, tag="bq", name="bq_sb")
        bk_sb = singles.tile([128, HC], F32, tag="bk", name="bk_sb")
        mk_sb = singles.tile([128, SC], F32, tag="mk", name="mk_sb")
        ones_sb = singles.tile([128, 1], BF16, tag="ones", name="ones_sb")
        nc.gpsimd.dma_start(out=bq_sb[:], in_=bq.ap().rearrange("(j p) -> p j", p=128))
        nc.gpsimd.dma_start(out=bk_sb[:], in_=bk.ap().rearrange("(j p) -> p j", p=128))
        nc.gpsimd.dma_start(out=mk_sb[:], in_=mk.ap().rearrange("(j p) -> p j", p=128))
        nc.vector.memset(ones_sb[:], 1.0)

        # k^T, laid out [d, kp]; stays resident for the whole kernel
        kt = [kt_pool.tile([128, S], F32R, tag=f"kt{d}", name=f"kt{d}") for d in range(HC)]

        # PE warm-up: keep the PE ticking through the initial DMA wait so the
        # HAM clock-gate opens before the first real matmul.
        with tc.tile_pool(name="psw", bufs=1, space="PSUM") as psw:
            pjunk = psw.tile([128, 512], F32, tag="pj", name="pj")
            for _ in range(48):
                nc.tensor.matmul(
                    pjunk[:], lhsT=junk[:, 0:128], rhs=junk[:], start=True, stop=True
                )

        # ---- Phase A: projections. q^T spills to DRAM, k^T stays in SBUF.
        with ExitStack() as pa:
            wt_pool = pa.enter_context(tc.tile_pool(name="wtp", bufs=1))
            hst_pool = pa.enter_context(tc.tile_pool(name="hstp", bufs=2))
            psA = pa.enter_context(tc.tile_pool(name="psA", bufs=8, space="PSUM"))
            qb_pool = pa.enter_context(tc.tile_pool(name="qbp", bufs=4))

            # Both weight matrices resident; hs^T streamed in 512-wide column
            # blocks so the first matmuls start after ~1MB of DMA, not 12MB.
            wk_sb = [wt_pool.tile([128, H], F32R, tag=f"wk{h}", name=f"wk{h}") for h in range(HC)]
            wq_sb = [wt_pool.tile([128, H], F32R, tag=f"wq{h}", name=f"wq{h}") for h in range(HC)]
            hs0 = []
            for h in range(HC):
                nc.sync.dma_start(out=wk_sb[h][:], in_=wkT.ap()[h * 128:(h + 1) * 128, :])
                t = hst_pool.tile([128, 512], F32R, tag=f"hst{h}", name=f"hst{h}")
                nc.sync.dma_start(out=t[:], in_=hsT.ap()[h * 128:(h + 1) * 128, 0:512])
                hs0.append(t)
            for h in range(HC):
                nc.sync.dma_start(out=wq_sb[h][:], in_=wqT.ap()[h * 128:(h + 1) * 128, :])

            for sb in range(SB):
                if sb == 0:
                    hsc = hs0
                else:
                    hsc = []
                    for h in range(HC):
                        t = hst_pool.tile([128, 512], F32R, tag=f"hst{h}", name=f"hst{h}")
                        nc.sync.dma_start(
                            out=t[:],
                            in_=hsT.ap()[h * 128:(h + 1) * 128, sb * 512:(sb + 1) * 512],
                        )
                        hsc.append(t)
                for w_sb, bias_sb, is_q in ((wk_sb, bk_sb, False), (wq_sb, bq_sb, True)):
                    for oc in range(HC):
                        ps = psA.tile([128, 512], F32, tag="psA", name="psa")
                        for h in range(HC):
                            nc.tensor.matmul(
                                ps[:],
                                lhsT=w_sb[h][:, oc * 128:(oc + 1) * 128],
                                rhs=hsc[h][:],
                                start=(h == 0),
                                stop=(h == HC - 1),
                            )
                        if is_q:
                            qb = qb_pool.tile([128, 512], F32R, tag="qb", name="qb")
                            nc.scalar.activation(
                                out=qb[:], in_=ps[:],
                                func=mybir.ActivationFunctionType.Identity,
                                bias=bias_sb[:, oc:oc + 1],
                            )
                            nc.scalar.dma_start(
                                out=qts[sb].ap()[oc * 128:(oc + 1) * 128, :],
                                in_=qb[:],
                            )
                        else:
                            nc.scalar.activation(
                                out=kt[oc][:, sb * 512:(sb + 1) * 512], in_=ps[:],
                                func=mybir.ActivationFunctionType.Identity,
                                bias=bias_sb[:, oc:oc + 1],
                            )

        # ---- Phase B: scores^T -> exp/mask -> attention-value, per 512-wide
        # block of query positions.
        with ExitStack() as pb:
            hsb_pool = pb.enter_context(tc.tile_pool(name="hsbp", bufs=1))
            qt_pool = pb.enter_context(tc.tile_pool(name="qtp", bufs=2))
            et_pool = pb.enter_context(tc.tile_pool(name="etp", bufs=1))
            ps_s = pb.enter_context(tc.tile_pool(name="pss", bufs=2, space="PSUM"))
            ps_o = pb.enter_context(tc.tile_pool(name="pso", bufs=2, space="PSUM"))
            ps_n = pb.enter_context(tc.tile_pool(name="psn", bufs=1, space="PSUM"))
            out_pool = pb.enter_context(tc.tile_pool(name="outp", bufs=2))
            r_pool = pb.enter_context(tc.tile_pool(name="rp", bufs=4))

            psw2 = pb.enter_context(tc.tile_pool(name="psw2", bufs=1, space="PSUM"))
            pjunk2 = psw2.tile([128, 512], F32, tag="pj2", name="pj2")
            for _ in range(24):
                nc.tensor.matmul(
                    pjunk2[:], lhsT=junk[:, 0:128], rhs=junk[:], start=True, stop=True
                )

            hsbt = [hsb_pool.tile([128, H], BF16, tag=f"hsb{k}", name=f"hsb{k}") for k in range(SC)]

            for b in range(SB):
                qtb = [qt_pool.tile([128, 512], F32R, tag=f"qt{d}", name=f"qt{d}") for d in range(HC)]
                for d in range(HC):
                    nc.sync.dma_start(
                        out=qtb[d][:],
                        in_=qts[b].ap()[d * 128:(d + 1) * 128, :],
                    )
                if b == 0:
                    for k in range(SC):
                        nc.scalar.dma_start(out=hsbt[k][:], in_=hsb.ap()[k * 128:(k + 1) * 128, :])
                et = [et_pool.tile([128, 512], BF16, tag=f"et{k}", name=f"et{k}") for k in range(SC)]
                for k in range(SC):
                    ps = ps_s.tile([128, 512], F32, tag="pss", name="pss")
                    for d in range(HC):
                        nc.tensor.matmul(
                            ps[:],
                            lhsT=kt[d][:, k * 128:(k + 1) * 128],
                            rhs=qtb[d][:],
                            start=(d == 0),
                            stop=(d == HC - 1),
                        )
                    nc.scalar.activation(
                        out=et[k][:], in_=ps[:],
                        func=mybir.ActivationFunctionType.Exp,
                        scale=1.0 / 32.0,
                    )
                    nc.vector.tensor_scalar_mul(
                        out=et[k][:], in0=et[k][:], scalar1=mk_sb[:, k:k + 1]
                    )
                for qs in range(4):
                    po0 = ps_o.tile([128, 512], F32, tag="po0", name="po0")
                    po1 = ps_o.tile([128, 512], F32, tag="po1", name="po1")
                    pn = ps_n.tile([128, 1], F32, tag="pn", name="pn")
                    for k in range(SC):
                        lw = et[k][:, qs * 128:(qs + 1) * 128]
                        st, sp = (k == 0), (k == SC - 1)
                        nc.tensor.matmul(po0[:], lhsT=lw, rhs=hsbt[k][:, 0:512], start=st, stop=sp)
                        nc.tensor.matmul(po1[:], lhsT=lw, rhs=hsbt[k][:, 512:1024], start=st, stop=sp)
                        nc.tensor.matmul(pn[:], lhsT=lw, rhs=ones_sb[:], start=st, stop=sp)
                    r = r_pool.tile([128, 1], F32, tag="r", name="r")
                    nc.vector.reciprocal(r[:], pn[:, 0:1])
                    ot = out_pool.tile([128, H], F32, tag="ot", name="ot")
                    nc.vector.tensor_scalar_mul(out=ot[:, 0:512], in0=po0[:], scalar1=r[:])
                    nc.vector.tensor_scalar_mul(out=ot[:, 512:1024], in0=po1[:], scalar1=r[:])
                    row = b * 512 + qs * 128
                    nc.scalar.dma_start(out=out.ap()[row:row + 128, :], in_=ot[:])

    nc.finalize()
    return nc


def kernel(hidden_states, key_padding_mask, Wq_w, Wq_b, Wk_w, Wk_b):
    global _CACHED_NC
    if _CACHED_NC is None:
        _CACHED_NC = build_nc()
    nc = _CACHED_NC

    hs = np.ascontiguousarray(hidden_states, dtype=np.float32)
    wqT = np.ascontiguousarray(np.asarray(Wq_w, dtype=np.float32).T)
    wkT = np.ascontiguousarray(np.asarray(Wk_w, dtype=np.float32).T)
    bq = np.ascontiguousarray(Wq_b, dtype=np.float32)
    bk = np.ascontiguousarray(Wk_b, dtype=np.float32)
    keep = (~np.asarray(key_padding_mask, dtype=bool)).astype(np.float32)

    in_maps = []
    for b in range(B):
        in_maps.append({
            "hst": np.ascontiguousarray(hs[b].T),
            "hsb": hs[b].astype(ml_dtypes.bfloat16),
            "wqt": wqT,
            "wkt": wkT,
            "bq": bq,
            "bk": bk,
            "mk": np.ascontiguousarray(keep[b]),
        })

    res = run_bass_kernel_spmd(nc, in_maps, core_ids=list(range(N_CORES)))
    return np.stack([res.results[b]["out"] for b in range(B)]).astype(np.float32)


# revision 13
# speedup vs baseline: 1.0310x; 1.0310x over previous
"""Trainium2 Bass kernel for AttnNoProjVal.

Computes, per batch element b (one NeuronCore per batch element, B=8):
  q = hs @ Wq.T + bq ; k = hs @ Wk.T + bk
  scoresT[kp, qp] = k[kp] . q[qp] / sqrt(H)      (transposed orientation)
  E = exp(scoresT) * keep_mask[kp]               (no row-max: |scores| <~ 8)
  out[qp, :] = (E.T @ hs) / colsum(E)[qp]

The transposed score orientation puts the softmax reduction on the PE
partition axis, where it folds into the attention-value matmul as an extra
N=1 column of ones -- no transposes and no vector-engine reductions needed.
All matmuls run as float32r (full PE rate at moving-dim >= 256); the
attention-value matmul runs in bf16.
"""

import sys

sys.path.insert(0, "/opt/trn_rl_repo")

from contextlib import ExitStack

import ml_dtypes
import numpy as np

import concourse.bass as bass
import concourse.tile as tile
from concourse import bacc, mybir
from concourse.bass_utils import run_bass_kernel_spmd

B, S, H = 8, 2048, 1024
N_CORES = 8
HC = H // 128   # 8 chunks of the hidden/head dim
SC = S // 128   # 16 chunks of the sequence dim
SB = S // 512   # 4 moving-dim blocks of the sequence dim
F32 = mybir.dt.float32
F32R = mybir.dt.float32r
BF16 = mybir.dt.bfloat16

_CACHED_NC = None


def _r(ap):
    return ap.bitcast(F32R)


def build_nc():
    nc = bacc.Bacc(None, target_bir_lowering=False)

    hsT = nc.dram_tensor("hst", [H, S], F32R, kind="ExternalInput")
    hsb = nc.dram_tensor("hsb", [S, H], BF16, kind="ExternalInput")
    wqT = nc.dram_tensor("wqt", [H, H], F32R, kind="ExternalInput")
    wkT = nc.dram_tensor("wkt", [H, H], F32R, kind="ExternalInput")
    bq = nc.dram_tensor("bq", [H], F32, kind="ExternalInput")
    bk = nc.dram_tensor("bk", [H], F32, kind="ExternalInput")
    mk = nc.dram_tensor("mk", [S], F32, kind="ExternalInput")
    out = nc.dram_tensor("out", [S, H], F32, kind="ExternalOutput")
    # Per-block DRAM spill tensors for q^T: block b's reload in phase B only
    # depends on its own 8 spill writes, so it overlaps the rest of phase A.
    qts = [nc.dram_tensor(f"qts{b}", [H, 512], F32R) for b in range(SB)]

    with tile.TileContext(nc) as tc, ExitStack() as whole:
        singles = whole.enter_context(tc.tile_pool(name="singles", bufs=1))
        kt_pool = whole.enter_context(tc.tile_pool(name="ktp", bufs=1))

        junk = singles.tile([128, 512], BF16, tag="junk", name="junk")
        nc.vector.memset(junk[:], 0.0)
        bq_sb = singles.tile([128, HC], F32, tag="bq", name="bq_sb")
        bk_sb = singles.tile([128, HC], F32, tag="bk", name="bk_sb")
        mk_sb = singles.tile([128, SC], F32, tag="mk", name="mk_sb")
        ones_sb = singles.tile([128, 1], BF16, tag="ones", name="ones_sb")
        nc.gpsimd.dma_start(out=bq_sb[:], in_=bq.ap().rearrange("(j p) -> p j", p=128))
        nc.gpsimd.dma_start(out=bk_sb[:], in_=bk.ap().rearrange("(j p) -> p j", p=128))
        nc.gpsimd.dma_start(out=mk_sb[:], in_=mk.ap().rearrange("(j p) -> p j", p=128))
        nc.vector.memset(ones_sb[:], 1.0)

        # k^T, laid out [d, kp]; stays resident for the whole kernel
        kt = [kt_pool.tile([128, S], F32R, tag=f"kt{d}", name=f"kt{d}") for d in range(HC)]

        # PE warm-up: keep the PE ticking through the initial DMA wait so the
        # HAM clock-gate opens before the first real matmul.
        with tc.tile_pool(name="psw", bufs=1, space="PSUM") as psw:
            pjunk = psw.tile([128, 512], F32, tag="pj", name="pj")
            for _ in range(48):
                nc.tensor.matmul(
                    pjunk[:], lhsT=junk[:, 0:128], rhs=junk[:], start=True, stop=True
                )

        # ---- Phase A: projections. q^T spills to DRAM, k^T stays in SBUF.
        with ExitStack() as pa:
            wt_pool = pa.enter_context(tc.tile_pool(name="wtp", bufs=1))
            hst_pool = pa.enter_context(tc.tile_pool(name="hstp", bufs=2))
            psA = pa.enter_context(tc.tile_pool(name="psA", bufs=8, space="PSUM"))
            qb_pool = pa.enter_context(tc.tile_pool(name="qbp", bufs=4))

            # Both weight matrices resident; hs^T streamed in 512-wide column
            # blocks so the first matmuls start after ~1MB of DMA, not 12MB.
            wk_sb = [wt_pool.tile([128, H], F32R, tag=f"wk{h}", name=f"wk{h}") for h in range(HC)]
            wq_sb = [wt_pool.tile([128, H], F32R, tag=f"wq{h}", name=f"wq{h}") for h in range(HC)]
            hs0 = []
            for h in range(HC):
                nc.sync.dma_start(out=wk_sb[h][:], in_=wkT.ap()[h * 128:(h + 1) * 128, :])
                t = hst_pool.tile([128, 512], F32R, tag=f"hst{h}", name=f"hst{h}")
                nc.sync.dma_start(out=t[:], in_=hsT.ap()[h * 128:(h + 1) * 128, 0:512])
                hs0.append(t)
            for h in range(HC):
                nc.sync.dma_start(out=wq_sb[h][:], in_=wqT.ap()[h * 128:(h + 1) * 128, :])

            for sb in range(SB):
                if sb == 0:
                    hsc = hs0
                else:
                    hsc = []
                    for h in range(HC):
                        t = hst_pool.tile([128, 512], F32R, tag=f"hst{h}", name=f"hst{h}")
                        nc.sync.dma_start(
                            out=t[:],
                            in_=hsT.ap()[h * 128:(h + 1) * 128, sb * 512:(sb + 1) * 512],
                        )
                        hsc.append(t)
                for w_sb, bias_sb, is_q in ((wk_sb, bk_sb, False), (wq_sb, bq_sb, True)):
                    for oc in range(HC):
                        ps = psA.tile([128, 512], F32, tag="psA", name="psa")
                        for h in range(HC):
                            nc.tensor.matmul(
                                ps[:],
                                lhsT=w_sb[h][:, oc * 128:(oc + 1) * 128],
                                rhs=hsc[h][:],
                                start=(h == 0),
                                stop=(h == HC - 1),
                            )
                        if is_q:
                            qb = qb_pool.tile([128, 512], F32R, tag="qb", name="qb")
                            nc.scalar.activation(
                                out=qb[:], in_=ps[:],
                                func=mybir.ActivationFunctionType.Identity,
                                bias=bias_sb[:, oc:oc + 1],
                            )
                            nc.scalar.dma_start(
                                out=qts[sb].ap()[oc * 128:(oc + 1) * 128, :],
                                in_=qb[:],
                            )
                        else:
                            nc.scalar.activation(
                                out=kt[oc][:, sb * 512:(sb + 1) * 512], in_=ps[:],
                                func=mybir.ActivationFunctionType.Identity,
                                bias=bias_sb[:, oc:oc + 1],
                            )

        # ---- Phase B: scores^T -> exp/mask -> attention-value, per 512-wide
        # block of query positions.
        with ExitStack() as pb:
            hsb_pool = pb.enter_context(tc.tile_pool(name="hsbp", bufs=1))
            qt_pool = pb.enter_context(tc.tile_pool(name="qtp", bufs=2))
            et_pool = pb.enter_context(tc.tile_pool(name="etp", bufs=1))
            ps_s = pb.enter_context(tc.tile_pool(name="pss", bufs=2, space="PSUM"))
            ps_o = pb.enter_context(tc.tile_pool(name="pso", bufs=2, space="PSUM"))
            ps_n = pb.enter_context(tc.tile_pool(name="psn", bufs=1, space="PSUM"))
            out_pool = pb.enter_context(tc.tile_pool(name="outp", bufs=2))
            r_pool = pb.enter_context(tc.tile_pool(name="rp", bufs=4))

            psw2 = pb.enter_context(tc.tile_pool(name="psw2", bufs=1, space="PSUM"))
            pjunk2 = psw2.tile([128, 512], F32, tag="pj2", name="pj2")
            for _ in range(24):
                nc.tensor.matmul(
                    pjunk2[:], lhsT=junk[:, 0:128], rhs=junk[:], start=True, stop=True
                )

            hsbt = [hsb_pool.tile([128, H], BF16, tag=f"hsb{k}", name=f"hsb{k}") for k in range(SC)]

            for b in range(SB):
                qtb = [qt_pool.tile([128, 512], F32R, tag=f"qt{d}", name=f"qt{d}") for d in range(HC)]
                for d in range(HC):
                    nc.sync.dma_start(
                        out=qtb[d][:],
                        in_=qts[b].ap()[d * 128:(d + 1) * 128, :],
                    )
                if b == 0:
                    for k in range(SC):
                        nc.sync.dma_start(out=hsbt[k][:], in_=hsb.ap()[k * 128:(k + 1) * 128, :])
                et = [et_pool.tile([128, 512], BF16, tag=f"et{k}", name=f"et{k}") for k in range(SC)]
                for k in range(SC):
                    ps = ps_s.tile([128, 512], F32, tag="pss", name="pss")
                    for d in range(HC):
                        nc.tensor.matmul(
                            ps[:],
                            lhsT=kt[d][:, k * 128:(k + 1) * 128],
                            rhs=qtb[d][:],
                            start=(d == 0),
                            stop=(d == HC - 1),
                        )
                    nc.scalar.activation(
                        out=et[k][:], in_=ps[:],
                        func=mybir.ActivationFunctionType.Exp,
                        scale=1.0 / 32.0,
                    )
                    nc.vector.tensor_scalar_mul(
                        out=et[k][:], in0=et[k][:], scalar1=mk_sb[:, k:k + 1]
                    )
                for qs in range(4):
                    po0 = ps_o.tile([128, 512], F32, tag="po0", name="po0")
                    po1 = ps_o.tile([128, 512], F32, tag="po1", name="po1")
                    pn = ps_n.tile([128, 1], F32, tag="pn", name="pn")
                    for k in range(SC):
                        lw = et[k][:, qs * 128:(qs + 1) * 128]
                        st, sp = (k == 0), (k == SC - 1)
                        nc.tensor.matmul(po0[:], lhsT=lw, rhs=hsbt[k][:, 0:512], start=st, stop=sp)
                        nc.tensor.matmul(po1[:], lhsT=lw, rhs=hsbt[k][:, 512:1024], start=st, stop=sp)
                        nc.tensor.matmul(pn[:], lhsT=lw, rhs=ones_sb[:], start=st, stop=sp)
                    r = r_pool.tile([128, 1], F32, tag="r", name="r")
                    nc.vector.reciprocal(r[:], pn[:, 0:1])
                    ot = out_pool.tile([128, H], F32, tag="ot", name="ot")
                    nc.vector.tensor_scalar_mul(out=ot[:, 0:512], in0=po0[:], scalar1=r[:])
                    nc.vector.tensor_scalar_mul(out=ot[:, 512:1024], in0=po1[:], scalar1=r[:])
                    row = b * 512 + qs * 128
                    nc.scalar.dma_start(out=out.ap()[row:row + 128, :], in_=ot[:])

    nc.finalize()
    return nc


def kernel(hidden_states, key_padding_mask, Wq_w, Wq_b, Wk_w, Wk_b):
    global _CACHED_NC
    if _CACHED_NC is None:
        _CACHED_NC = build_nc()
    nc = _CACHED_NC

    hs = np.ascontiguousarray(hidden_states, dtype=np.float32)
    wqT = np.ascontiguousarray(np.asarray(Wq_w, dtype=np.float32).T)
    wkT = np.ascontiguousarray(np.asarray(Wk_w, dtype=np.float32).T)
    bq = np.ascontiguousarray(Wq_b, dtype=np.float32)
    bk = np.ascontiguousarray(Wk_b, dtype=np.float32)
    keep = (~np.asarray(key_padding_mask, dtype=bool)).astype(np.float32)

    in_maps = []
    for b in range(B):
        in_maps.append({
            "hst": np.ascontiguousarray(hs[b].T),
            "hsb": hs[b].astype(ml_dtypes.bfloat16),
            "wqt": wqT,
            "wkt": wkT,
            "bq": bq,
            "bk": bk,
            "mk": np.ascontiguousarray(keep[b]),
        })

    res = run_bass_kernel_spmd(nc, in_maps, core_ids=list(range(N_CORES)))
    return np.stack([res.results[b]["out"] for b in range(B)]).astype(np.float32)
